# revision 9
# baseline (speedup 1.0000x reference)
import os
import sys

sys.path.insert(0, "/opt/trn_rl_repo")

import numpy as np
import ml_dtypes

import concourse.bass as bass
from concourse import bacc
import concourse.mybir as mybir
import concourse.tile as tile
from concourse.bass_utils import run_bass_kernel_spmd

F32 = mybir.dt.float32
BF16 = mybir.dt.bfloat16
AF = mybir.ActivationFunctionType

N_CORES = 8
B, S, I, HS = 128, 1024, 256, 256
BS = B // N_CORES  # 16 batch rows per core
G = 4 * HS  # 1024 gate columns
NGT = G // 128  # 8 gate tiles of 128 cols
KC = 2  # contraction chunks of 128 (for both I and HS)
TB = 64  # timesteps per pipeline block
BF16H = bool(int(os.environ.get("BASS_LSTM_BF16H", "0")))

# permuted gate order [i, f, o, g] so sigmoid gates are contiguous
_PERM = np.concatenate(
    [np.arange(0, 512), np.arange(768, 1024), np.arange(512, 768)]
)

_BUILD_CACHE = {}


def _build(seq_len, tb):
    """Emit + compile the SPMD program (one NeuronCore; same code all cores)."""
    nb_blocks = seq_len // tb
    nc = bacc.Bacc("TRN2", target_bir_lowering=False, debug=False)

    xt = nc.dram_tensor("xt", [I, seq_len * BS], F32, kind="ExternalInput")
    w_in = nc.dram_tensor("w_in", [128, KC * G], F32, kind="ExternalInput")
    u_in = nc.dram_tensor("u_in", [128, KC * G], BF16, kind="ExternalInput")
    b_in = nc.dram_tensor("b_in", [128, NGT], F32, kind="ExternalInput")
    id_in = nc.dram_tensor("id_in", [128, 128], F32, kind="ExternalInput")
    out_h = nc.dram_tensor("out_h", [128, seq_len * 2 * BS], F32, kind="ExternalOutput")
    out_c = nc.dram_tensor("out_c", [128, 2 * BS], F32, kind="ExternalOutput")

    with tile.TileContext(nc) as tc:
        with (
            tc.tile_pool(name="const", bufs=1) as const,
            tc.tile_pool(name="xp", bufs=2) as xp,
            tc.tile_pool(name="xwtp", bufs=2) as xwtp,
            tc.tile_pool(name="pc_ps", bufs=4, space="PSUM") as pc_ps,
            tc.tile_pool(name="rec_ps", bufs=1, space="PSUM") as rec_ps,
            tc.tile_pool(name="gsp", bufs=3) as gsp,
            tc.tile_pool(name="cp", bufs=2) as cp,
            tc.tile_pool(name="igp", bufs=2) as igp,
            tc.tile_pool(name="tctp", bufs=2) as tctp,
            tc.tile_pool(name="hbfp", bufs=2) as hbfp,
            tc.tile_pool(name="outp", bufs=2) as outp,
        ):
            w_sb = const.tile([128, KC * G], F32, tag="w_sb")
            u_sb = const.tile([128, KC * G], BF16, tag="u_sb")
            bias_sb = const.tile([128, NGT], F32, tag="bias_sb")
            id_sb = const.tile([128, 128], F32, tag="id_sb")
            nc.sync.dma_start(w_sb[:], w_in[:])
            nc.sync.dma_start(u_sb[:], u_in[:])
            nc.sync.dma_start(bias_sb[:], b_in[:])
            nc.sync.dma_start(id_sb[:], id_in[:])

            # initial state h=0, c=0
            c_prev = cp.tile([128, 2 * BS], F32, tag="c")
            nc.vector.memset(c_prev[:], 0.0)
            hbf_prev = hbfp.tile([128, 2 * BS], BF16, tag="hbf")
            nc.vector.memset(hbf_prev[:], 0.0)

            def load_x_block(nbi):
                t0 = nbi * tb
                xt_tile = xp.tile([128, KC * tb * BS], F32, tag="xt")
                for kc in range(KC):
                    nc.sync.dma_start(
                        xt_tile[:, kc * tb * BS : (kc + 1) * tb * BS],
                        xt[kc * 128 : (kc + 1) * 128, t0 * BS : (t0 + tb) * BS],
                    )
                return xt_tile

            def pc_unit(xt_tile, xwt_tile, gt, nch):
                # xW^T for 32 timesteps x 16 batch (512 cols), one gate tile
                ps = pc_ps.tile([128, 512], F32, tag="pcps")
                for kc in range(KC):
                    nc.tensor.matmul(
                        ps[:],
                        w_sb[:, kc * G + gt * 128 : kc * G + (gt + 1) * 128],
                        xt_tile[:, kc * tb * BS + nch * 512 : kc * tb * BS + (nch + 1) * 512],
                        start=(kc == 0),
                        stop=(kc == KC - 1),
                    )
                # scatter into xwt block (+bias): dst cols (t_l, gt, b)
                xwt3 = xwt_tile.rearrange("p (t c) -> p t c", c=128)
                nt = 512 // BS  # 32 timesteps per chunk
                dst = xwt3[:, nch * nt : (nch + 1) * nt, gt * BS : (gt + 1) * BS]
                nc.scalar.activation(
                    dst, ps[:], AF.Identity, bias=bias_sb[:, gt : gt + 1]
                )

            def make_pc_work(nbi, xwt_tile):
                work = []
                xt_holder = {}

                def load():
                    xt_holder["t"] = load_x_block(nbi)

                work.append(load)
                for gt in range(NGT):
                    for nch in range(tb * BS // 512):
                        work.append(
                            lambda gt=gt, nch=nch: pc_unit(
                                xt_holder["t"], xwt_tile, gt, nch
                            )
                        )
                return work

            # gate tile order: f first (unblocks c-chain), then g, i, o
            MM_ORDER = [2, 3, 6, 7, 0, 1, 4, 5]

            def step(xwt_tile, t_l, out_tile, c_prev, hbf_prev):
                # one PSUM bank per gate so each bank's accumulation group
                # closes before any engine reads it (PE-W + read same bank
                # is fatal on HW). Bank order i,g,f,o: the c-chain
                # (sig_i -> i*g -> add -> tanh_c) is the longest dependency
                # chain, so close i and g first.
                psi = rec_ps.tile([128, 2 * BS], F32, tag="psi")
                psg = rec_ps.tile([128, 2 * BS], F32, tag="psg")
                psf = rec_ps.tile([128, 2 * BS], F32, tag="psf")
                pso = rec_ps.tile([128, 2 * BS], F32, tag="pso")
                base = t_l * 128
                # init psum with xW_t + bias via identity matmul (N=32 each);
                # all four share one identity LDWEIGHTS and can run during the
                # previous step's tail (they don't depend on h)
                nc.tensor.matmul(psi[:], id_sb[:], xwt_tile[:, base : base + 32], start=True, stop=False)
                nc.tensor.matmul(psg[:], id_sb[:], xwt_tile[:, base + 96 : base + 128], start=True, stop=False)
                nc.tensor.matmul(psf[:], id_sb[:], xwt_tile[:, base + 32 : base + 64], start=True, stop=False)
                nc.tensor.matmul(pso[:], id_sb[:], xwt_tile[:, base + 64 : base + 96], start=True, stop=False)

                gs = gsp.tile([128, 128], F32, tag="gs")
                cnew = cp.tile([128, 2 * BS], F32, tag="c")
                ig = igp.tile([128, 2 * BS], F32, tag="ig")
                tct = tctp.tile([128, 2 * BS], F32, tag="tct")
                hbf = hbfp.tile([128, 2 * BS], BF16, tag="hbf")

                def mm_bank(pst, gts):
                    for j, gt in enumerate(gts):
                        for kc in range(KC):
                            nc.tensor.matmul(
                                pst[:, j * BS : (j + 1) * BS],
                                u_sb[:, kc * G + gt * 128 : kc * G + (gt + 1) * 128],
                                hbf_prev[:, kc * BS : (kc + 1) * BS],
                                start=False,
                                stop=(j == 1 and kc == KC - 1),
                            )

                mm_bank(psi, (0, 1))  # i
                nc.scalar.activation(gs[:, 0:32], psi[:], AF.Sigmoid)
                mm_bank(psg, (6, 7))  # g
                nc.scalar.activation(gs[:, 96:128], psg[:], AF.Tanh)
                nc.vector.tensor_mul(ig[:], gs[:, 0:32], gs[:, 96:128])  # i*g
                mm_bank(psf, (2, 3))  # f
                nc.scalar.activation(gs[:, 32:64], psf[:], AF.Sigmoid)
                nc.vector.tensor_mul(cnew[:], gs[:, 32:64], c_prev[:])  # f*c
                nc.vector.tensor_add(cnew[:], cnew[:], ig[:])
                mm_bank(pso, (4, 5))  # o
                nc.scalar.activation(gs[:, 64:96], pso[:], AF.Sigmoid)
                nc.scalar.activation(tct[:], cnew[:], AF.Tanh)
                hs_dst = out_tile[:, t_l * 2 * BS : (t_l + 1) * 2 * BS]
                if BF16H:
                    # bf16 h first (unblocks next step's matmuls), fp32 copy after
                    nc.vector.tensor_mul(hbf[:], gs[:, 64:96], tct[:])
                    nc.vector.tensor_copy(hs_dst, hbf[:])
                else:
                    nc.vector.tensor_mul(hs_dst, gs[:, 64:96], tct[:])  # h = o*tanh(c)
                    nc.vector.tensor_copy(hbf[:], hs_dst)  # bf16 cast for next matmul
                return cnew, hbf

            # prologue: block 0 precompute fully
            xwt_tiles = {}
            xwt_tiles[0] = xwtp.tile([128, tb * 128], F32, tag="xwt", name="xwt0")
            for fn in make_pc_work(0, xwt_tiles[0]):
                fn()

            for nbi in range(nb_blocks):
                out_tile = outp.tile([128, tb * 2 * BS], F32, tag="outt")
                pc_work = []
                if nbi + 1 < nb_blocks:
                    xwt_tiles[nbi + 1] = xwtp.tile([128, tb * 128], F32, tag="xwt", name=f"xwt{nbi + 1}")
                    pc_work = make_pc_work(nbi + 1, xwt_tiles[nbi + 1])
                # spread precompute work over this block's steps
                per = {}
                if pc_work:
                    stride = max(1, tb // len(pc_work))
                    for wi, fn in enumerate(pc_work):
                        per.setdefault(min(wi * stride, tb - 1), []).append(fn)
                for t_l in range(tb):
                    c_prev, hbf_prev = step(
                        xwt_tiles[nbi], t_l, out_tile, c_prev, hbf_prev
                    )
                    for fn in per.get(t_l, ()):
                        fn()
                xwt_tiles.pop(nbi)
                nc.sync.dma_start(
                    out_h[:, nbi * tb * 2 * BS : (nbi + 1) * tb * 2 * BS], out_tile[:]
                )

            nc.sync.dma_start(out_c[:], c_prev[:])

    nc.compile()
    return nc


def _get_nc(seq_len, tb):
    key = (seq_len, tb)
    if key not in _BUILD_CACHE:
        _BUILD_CACHE[key] = _build(seq_len, tb)
    return _BUILD_CACHE[key]


def _prep_shared(W, U, bias):
    Wp = np.ascontiguousarray(W[:, _PERM], dtype=np.float32)
    Up = np.ascontiguousarray(U[:, _PERM], dtype=np.float32)
    bp = np.ascontiguousarray(bias[_PERM], dtype=np.float32)
    w_host = Wp.reshape(KC, 128, G).transpose(1, 0, 2).reshape(128, KC * G)
    u_host = (
        Up.reshape(KC, 128, G)
        .transpose(1, 0, 2)
        .reshape(128, KC * G)
        .astype(ml_dtypes.bfloat16)
    )
    b_host = bp.reshape(NGT, 128).T
    ident = np.eye(128, dtype=np.float32)
    return (
        np.ascontiguousarray(w_host),
        np.ascontiguousarray(u_host),
        np.ascontiguousarray(b_host),
        ident,
    )


def _make_in_maps(x, W, U, bias):
    seq_len = x.shape[1]
    w_host, u_host, b_host, ident = _prep_shared(W, U, bias)
    in_maps = []
    for core in range(N_CORES):
        xs = x[core * BS : (core + 1) * BS]  # [BS, S, I]
        xt_host = np.ascontiguousarray(
            xs.transpose(2, 1, 0).reshape(I, seq_len * BS)
        )
        in_maps.append(
            {
                "xt": xt_host,
                "w_in": w_host,
                "u_in": u_host,
                "b_in": b_host,
                "id_in": ident,
            }
        )
    return in_maps


def kernel(x, W, U, bias):
    x = np.asarray(x, dtype=np.float32)
    W = np.asarray(W, dtype=np.float32)
    U = np.asarray(U, dtype=np.float32)
    bias = np.asarray(bias, dtype=np.float32)
    seq_len = x.shape[1]
    nc = _get_nc(seq_len, TB)
    in_maps = _make_in_maps(x, W, U, bias)

    res = run_bass_kernel_spmd(nc, in_maps, list(range(N_CORES)), trace=False)

    hidden = np.empty((B, seq_len, HS), dtype=np.float32)
    c_t = np.empty((B, HS), dtype=np.float32)
    for core in range(N_CORES):
        oh = res.results[core]["out_h"].reshape(128, seq_len, 2, BS)
        hidden[core * BS : (core + 1) * BS] = oh.transpose(3, 1, 2, 0).reshape(
            BS, seq_len, HS
        )
        oc = res.results[core]["out_c"].reshape(128, 2, BS)
        c_t[core * BS : (core + 1) * BS] = oc.transpose(2, 1, 0).reshape(BS, HS)
    h_t = hidden[:, -1].copy()
    return hidden, h_t, c_t
